# revision 27
# baseline (speedup 1.0000x reference)
"""MultiHeadAttention Trainium2 Bass kernel.

Problem: B=2, S=2048, D_MODEL=1024, H=16, D_K=64, f32.
  Q = inQ @ W_Q ; K = inK @ W_K ; V = inV @ W_V        [16 heads x d_k=64]
  scores = Q K^T / 8, masked (attn_mask -> -inf), attn = softmax(scores)
  context = attn @ V ; out = layer_norm(context @ W_fc + inQ)
  returns (out, attn)

Sharding: batch x query-rows across 8 cores. Core c handles batch c//4,
query rows [(c%4)*512, +512). Each core computes full K/V projections for
its batch (duplicated within the 4-core batch group), all 16 heads for its
512 query rows, then fc + layernorm for its rows. No collectives.

Per-core dataflow (matmuls in fp32r = full PE rate at N>=256):
  X^T via PE transposes -> K^T[feat,seq], V[seq,feat] (+ones col), Q^T[feat,q]
  A-path (context): S^T[k,q] = K^T-chunk.T @ Q^T (head-pair row-packed),
    + maskT (DVE add), expT = exp(x/8) (ACT, f32r out),
    ctxZ[65,q] += V_aug-chunk.T @ expT   (row 64 = softmax denominator Z)
  Z: recipZ row -> PE ones-broadcast -> normalize ctx; recipZ columns via
    N=1 matmuls; batched ACT Ln -> -lnZ per-q-partition bias
  C-path (attn out): S[q,k] = Q^T-chunk.T @ K^T (row-packed),
    attn = exp(S/8 - lnZ) (ACT per-partition bias) * m01 (GPSIMD) -> DMA rows
  fc: out = layer_norm(ctxT.T @ W_fc + inQ) via tensor_scalar.

SBUF: K^T fully resident; V processed in two head-halves (0-7 resident,
8-15 stashed to DRAM scratch during projections and reloaded).
Engine balance: exps on ACT, mask-add + copies on DVE, output mask-mult +
mask prep on GPSIMD, everything matmul on PE.
"""
import numpy as np
from contextlib import ExitStack

import concourse.bass as bass
import concourse.tile as tile
from concourse import bacc, mybir
from concourse.masks import make_identity

F32 = mybir.dt.float32
F32R = mybir.dt.float32r
BF16 = mybir.dt.bfloat16
U8 = mybir.dt.uint8

B, S, D, H, DK = 2, 2048, 1024, 16, 64
NCORES = 8
QN = (B * S) // NCORES          # 512 query rows per core
NQT = QN // 128                 # 4 q tiles of 128
EPS = 1e-5
MASKVAL = 200.0                 # subtracted from masked scores (pre /8)
AxX = mybir.AxisListType.X
Act = mybir.ActivationFunctionType
Alu = mybir.AluOpType
ds = bass.ds


def build_nc(reps=1, phases=(0, 1, 2, 3)):
    nc = bacc.Bacc("TRN2", target_bir_lowering=False, debug=False)
    XQ = nc.declare_dram_parameter("XQ", [QN, D], F32, isOutput=False)
    XK = nc.declare_dram_parameter("XK", [S, D], F32, isOutput=False)
    XV = nc.declare_dram_parameter("XV", [S, D], F32, isOutput=False)
    MASK = nc.declare_dram_parameter("MASKU8", [QN, S], U8, isOutput=False)
    WQ = nc.declare_dram_parameter("WQ", [D, D], F32R, isOutput=False)
    WK = nc.declare_dram_parameter("WK", [D, D], F32R, isOutput=False)
    WV = nc.declare_dram_parameter("WV", [D, D], F32R, isOutput=False)
    WFC = nc.declare_dram_parameter("WFC", [D, D], F32R, isOutput=False)
    OUT = nc.declare_dram_parameter("OUT", [QN, D], F32, isOutput=True)
    ATTN = nc.declare_dram_parameter("ATTN", [H, QN, S], F32, isOutput=True)
    SVA = nc.dram_tensor("SVA", [16, 128, 512], F32R)   # stash: V half B

    dram = dict(XQ=XQ, XK=XK, XV=XV, MASK=MASK, WQ=WQ, WK=WK, WV=WV,
                WFC=WFC, OUT=OUT, ATTN=ATTN, SVA=SVA)

    with tile.TileContext(nc) as tc, ExitStack() as ctx:
        const = ctx.enter_context(tc.tile_pool(name="const", bufs=1))
        id32 = const.tile([128, 128], F32, name="id32", tag="id32")
        make_identity(nc, id32[:])
        id16 = const.tile([128, 128], BF16, name="id16", tag="id16")
        make_identity(nc, id16[:])
        ones_f = const.tile([128, 128], F32, name="ones_f", tag="ones_f")
        nc.vector.memset(ones_f[:], 1.0)
        ones_r = const.tile([128, 128], F32R, name="ones_r", tag="ones_r")
        nc.vector.tensor_copy(ones_r[:], ones_f[:])
        epsb = const.tile([128, 1], F32, name="epsb", tag="epsb")
        nc.vector.memset(epsb[:], EPS)
        consts = dict(id32=id32, id16=id16, ones_f=ones_f, ones_r=ones_r,
                      epsb=epsb)

        persist = ctx.enter_context(tc.tile_pool(name="persist", bufs=1))
        kvpool = ctx.enter_context(tc.tile_pool(name="kv", bufs=1))

        if reps == 1:
            _body(nc, tc, persist, kvpool, consts, dram, phases)
        else:
            with tc.For_i(0, reps, 1):
                _body(nc, tc, persist, kvpool, consts, dram, phases)

    nc.compile()
    return nc


def _body(nc, tc, persist, kvpool, consts, dram, phases):
    id32, id16 = consts["id32"], consts["id16"]
    ones_f, ones_r, epsb = consts["ones_f"], consts["ones_r"], consts["epsb"]
    XQ, XK, XV, MASK = dram["XQ"], dram["XK"], dram["XV"], dram["MASK"]
    WQ, WK, WV, WFC = dram["WQ"], dram["WK"], dram["WV"], dram["WFC"]
    OUT, ATTN, SVA = dram["OUT"], dram["ATTN"], dram["SVA"]

    QT = [persist.tile([128, QN], F32R, name=f"QT{f}", tag=f"QT{f}")
          for f in range(8)]
    CTXT = [persist.tile([128, QN], F32R, name=f"CTXT{f}", tag=f"CTXT{f}")
            for f in range(8)]
    KT = [kvpool.tile([128, S], F32R, name=f"kt{f}", tag=f"kt{f}")
          for f in range(8)]
    VA = [kvpool.tile([128, 8 * 65], F32R, name=f"va{s}", tag=f"va{s}")
          for s in range(16)]

    def load_w(Wdram, pool):
        wr = [pool.tile([128, 1024], F32R, name=f"w{dm}", tag=f"w{dm}")
              for dm in range(8)]
        for dm in range(8):
            nc.sync.dma_start(wr[dm][:], Wdram[dm * 128:(dm + 1) * 128, :])
        return wr

    def stream_xt(Xdram, sc, pool, psum):
        xt = [pool.tile([128, 512], F32R, name=f"xt{dm}", tag=f"xt{dm}", bufs=2)
              for dm in range(8)]
        for st in range(4):
            xrow = pool.tile([128, 1024], F32, name="xrow", tag="xrow", bufs=2)
            nc.sync.dma_start(
                xrow[:], Xdram[sc * 512 + st * 128: sc * 512 + (st + 1) * 128, :])
            for dm in range(8):
                tp = psum.tile([128, 128], F32, name="xtp", tag="xtp")
                nc.tensor.transpose(tp[:], xrow[:, dm * 128:(dm + 1) * 128],
                                    id32[:])
                if dm % 2 == 0:
                    nc.scalar.copy(xt[dm][:, st * 128:(st + 1) * 128], tp[:])
                else:
                    nc.vector.tensor_copy(xt[dm][:, st * 128:(st + 1) * 128],
                                          tp[:])
        return xt

    # ---------------- P1: projections ----------------
    if 1 in phases:
        with tc.tile_pool(name="wkp", bufs=1) as wp, \
             tc.tile_pool(name="xs1", bufs=1) as xs, \
             tc.tile_pool(name="pp1", bufs=3, space="PSUM") as pp, \
             tc.tile_pool(name="px1", bufs=4, space="PSUM") as px:
            # K^T: all 8 feat tiles resident
            wk = load_w(WK, wp)
            for sc in range(4):
                xt = stream_xt(XK, sc, xs, px)
                for f in range(8):
                    ps = pp.tile([128, 512], F32, name="kps", tag="kps")
                    for dm in range(8):
                        nc.tensor.matmul(
                            ps[:], wk[dm][:, f * 128:(f + 1) * 128], xt[dm][:],
                            start=(dm == 0), stop=(dm == 7))
                    if f % 2 == 0:
                        nc.scalar.copy(KT[f][:, sc * 512:(sc + 1) * 512], ps[:])
                    else:
                        nc.vector.tensor_copy(
                            KT[f][:, sc * 512:(sc + 1) * 512], ps[:])

            # V: heads 0-7 resident (+ones), heads 8-15 stashed raw
            wv = load_w(WV, wp)
            for st16 in range(16):
                va_v = VA[st16][:].rearrange("p (h d) -> p h d", d=65)
                nc.vector.tensor_copy(
                    va_v[:, :, 64:65],
                    ones_f[:, 0:8].rearrange("p (h o) -> p h o", o=1))
            for sc in range(4):
                xt = stream_xt(XV, sc, xs, px)
                for st in range(4):
                    st16 = sc * 4 + st
                    va_v = VA[st16][:].rearrange("p (h d) -> p h d", d=65)
                    for fc2 in range(2):
                        ps = pp.tile([128, 512], F32, name="vps", tag="kps")
                        for dm in range(8):
                            nc.tensor.matmul(
                                ps[:], xt[dm][:, st * 128:(st + 1) * 128],
                                wv[dm][:, fc2 * 512:(fc2 + 1) * 512],
                                start=(dm == 0), stop=(dm == 7))
                        if fc2 == 0:
                            psv = ps[:].rearrange("p (h d) -> p h d", d=64)
                            nc.scalar.copy(va_v[:, 0:8, 0:64], psv[:, :, :])
                        else:
                            stg = xs.tile([128, 512], F32R, name="vstash",
                                          tag="vstash", bufs=2)
                            nc.vector.tensor_copy(stg[:], ps[:])
                            nc.sync.dma_start(SVA[st16], stg[:])

            # Q^T (all heads)
            wq = load_w(WQ, wp)
            xt = stream_xt(XQ, 0, xs, px)
            for f in range(8):
                ps = pp.tile([128, 512], F32, name="qps", tag="kps")
                for dm in range(8):
                    nc.tensor.matmul(
                        ps[:], wq[dm][:, f * 128:(f + 1) * 128], xt[dm][:],
                        start=(dm == 0), stop=(dm == 7))
                if f % 2 == 0:
                    nc.scalar.copy(QT[f][:, :], ps[:])
                else:
                    nc.vector.tensor_copy(QT[f][:, :], ps[:])

    # ---------------- P0 + P2 (mask pool scoped over both) ----------------
    if 0 not in phases:
        return
    with tc.tile_pool(name="maskp", bufs=1) as maskp:
        MADD = [maskp.tile([128, S], BF16, name=f"MADD{t}", tag=f"MADD{t}")
                for t in range(NQT)]
        MT = [maskp.tile([128, QN], BF16, name=f"MT{k}", tag=f"MT{k}")
              for k in range(16)]
        with tc.tile_pool(name="mprep", bufs=2) as mp, \
             tc.tile_pool(name="mpsum", bufs=4, space="PSUM") as mps:
            for qt in range(NQT):
                mu8 = mp.tile([128, S], U8, name="mu8", tag="mu8")
                nc.sync.dma_start(mu8[:], MASK[qt * 128:(qt + 1) * 128, :])
                # additive mask: 0 keep / -MASKVAL masked  (on GPSIMD)
                madd = MADD[qt]
                nc.gpsimd.tensor_scalar(
                    out=madd[:], in0=mu8[:], scalar1=-MASKVAL,
                    scalar2=0.0, op0=Alu.mult, op1=Alu.add)
                for kt in range(16):
                    tp = mps.tile([128, 128], BF16, name="tp", tag="tp")
                    with nc.allow_low_precision(reason="bf16 mask transpose"):
                        nc.tensor.transpose(
                            tp[:], madd[:, kt * 128:(kt + 1) * 128], id16[:])
                    nc.scalar.copy(MT[kt][:, qt * 128:(qt + 1) * 128], tp[:])

        # ---------------- P2: attention, two head-halves ----------------
        if 2 not in phases:
            return
        with tc.tile_pool(name="attnsb", bufs=2) as ab, \
             tc.tile_pool(name="zsb", bufs=1) as zb, \
             tc.tile_pool(name="azp", bufs=2, space="PSUM") as azp, \
             tc.tile_pool(name="cxp", bufs=2, space="PSUM") as cxp, \
             tc.tile_pool(name="spp", bufs=2, space="PSUM") as spp:
          for half in range(2):
            if half == 1:
                for st16 in range(16):
                    van = kvpool.tile([128, 8 * 65], F32R, name=f"va{st16}",
                                      tag=f"va{st16}")
                    va_v = van[:].rearrange("p (h d) -> p h d", d=65)
                    nc.sync.dma_start(va_v[:, :, 0:64], SVA[st16])
                    nc.vector.tensor_copy(
                        va_v[:, :, 64:65],
                        ones_f[:, 0:8].rearrange("p (h o) -> p h o", o=1))
                    VA[st16] = van

            nlz = {}

            def a_phase(hpl, half=half):
                hp = half * 4 + hpl
                ctxz = {}
                for hh in range(2):
                    ctxz[hh] = cxp.tile([65, QN], F32, name="ctxz", tag="ctxz")
                for kt in range(16):
                    msc = ab.tile([128, 2 * QN], F32, name="msc",
                                  tag="msc", bufs=2)
                    expt = ab.tile([128, 2 * QN], F32R, name="expt",
                                   tag="expt", bufs=2)
                    for hh in range(2):
                        hsl = ds(hh * 64, 64)
                        stps = azp.tile([128, QN], F32, name="az", tag="az")
                        nc.tensor.matmul(
                            stps[:],
                            KT[hp][hsl, kt * 128:(kt + 1) * 128],
                            QT[hp][hsl, :], start=True, stop=True,
                            tile_position=(hh * 64, 0))
                        nc.vector.tensor_tensor(
                            out=msc[:, ds(hh * QN, QN)],
                            in0=stps[:], in1=MT[kt][:, :], op=Alu.add)
                    nc.scalar.activation(expt[:], msc[:], Act.Exp, scale=0.125)
                    for hh in range(2):
                        hloc = 2 * hpl + hh
                        nc.tensor.matmul(
                            ctxz[hh][:],
                            VA[kt][:, hloc * 65:(hloc + 1) * 65],
                            expt[:, ds(hh * QN, QN)],
                            start=(kt == 0), stop=(kt == 15))
                # Z plumbing per head
                for hh in range(2):
                    hloc = 2 * hpl + hh
                    hsl = ds(hh * 64, 64)
                    rz = zb.tile([128, QN], F32R, name="rz", tag="rz")
                    with nc.allow_low_precision(reason="f32r recipZ"):
                        nc.vector.reciprocal(rz[64:65, :], ctxz[hh][64:65, :])
                    rzbc = azp.tile([128, QN], F32, name="az", tag="az")
                    nc.tensor.matmul(rzbc[:], ones_r[64:65, 0:128],
                                     rz[64:65, :], start=True, stop=True)
                    rzbc_sb = zb.tile([128, QN], F32, name="rzbc_sb",
                                      tag="rzbc_sb")
                    nc.vector.tensor_copy(rzbc_sb[:], rzbc[:])
                    nc.vector.tensor_tensor(
                        out=CTXT[hp][hsl, :], in0=ctxz[hh][0:64, :],
                        in1=rzbc_sb[0:64, :], op=Alu.mult)
                    # recipZ columns [128, NQT] via N=1 matmuls (bitcast to
                    # plain f32: fp32r has ISA restrictions at tiny sizes)
                    ztg = azp.tile([128, QN], F32, name="az", tag="az")
                    for j in range(NQT):
                        nc.tensor.matmul(
                            ztg[:, j:j + 1],
                            rz[64:65, j * 128:(j + 1) * 128].bitcast(F32),
                            ones_f[64:65, 0:1], start=True, stop=True)
                    zcol = zb.tile([128, NQT], F32, name=f"zcol{hloc}",
                                   tag=f"zcol{hloc}", bufs=2)
                    nc.vector.tensor_copy(zcol[:], ztg[:, 0:NQT])
                    nlz[hloc] = zcol

            def ln_batch(hlocs):
                for hloc in hlocs:
                    nl = zb.tile([128, NQT], F32, name=f"nl{hloc}",
                                 tag=f"nl{hloc}", bufs=2)
                    nc.scalar.activation(nl[:], nlz[hloc][:], Act.Ln)
                    nlz[hloc] = nl

            def c_phase(hpl, half=half):
                hp = half * 4 + hpl
                for j in range(NQT):
                    qtsl = ds(j * 128, 128)
                    stages = [ab.tile([128, S], F32, name="stage",
                                      tag="stage", bufs=2) for _ in range(2)]
                    for kc in range(2):
                        sps = {}
                        for hh in range(2):
                            hsl = ds(hh * 64, 64)
                            sps[hh] = spp.tile([128, 1024], F32, name="sps",
                                               tag="sps")
                            for k2 in range(2):
                                ksl = ds(kc * 1024 + k2 * 512, 512)
                                nc.tensor.matmul(
                                    sps[hh][:, k2 * 512:(k2 + 1) * 512],
                                    QT[hp][hsl, qtsl], KT[hp][hsl, ksl],
                                    start=True, stop=(k2 == 1),
                                    tile_position=(hh * 64, 0))
                            # += additive mask (0 / -MASKVAL), bf16 mms
                            for k2 in range(2):
                                nc.tensor.matmul(
                                    sps[hh][:, k2 * 512:(k2 + 1) * 512],
                                    id16[:],
                                    MADD[j][:, ds(kc * 1024 + k2 * 512, 512)],
                                    start=False, stop=True,
                                    skip_group_check=True)
                        for hh in range(2):
                            hloc = 2 * hpl + hh
                            nc.scalar.activation(
                                stages[hh][:, ds(kc * 1024, 1024)],
                                sps[hh][:], Act.Exp, scale=0.125,
                                bias=nlz[hloc][:, j:j + 1])
                    for hh in range(2):
                        h = 8 * half + 2 * hpl + hh
                        nc.sync.dma_start(
                            ATTN[h, j * 128:(j + 1) * 128, :], stages[hh][:])

            # two sub-batches of 2 pairs: A(01) Ln C(01) | A(23) Ln C(23)
            a_phase(0); a_phase(1)
            ln_batch([0, 1, 2, 3])
            c_phase(0); a_phase(2); c_phase(1); a_phase(3)
            ln_batch([4, 5, 6, 7])
            c_phase(2); c_phase(3)

    # ---------------- P3: fc + layernorm ----------------
    if 3 not in phases:
        return
    with tc.tile_pool(name="fcp", bufs=1) as fp, \
         tc.tile_pool(name="fcw", bufs=2) as fw, \
         tc.tile_pool(name="fps", bufs=4, space="PSUM") as fps:
        wfc = load_w(WFC, fp)
        inv_d = 1.0 / D
        for qt in range(NQT):
            xr = fw.tile([128, D], F32, name="xres", tag="xres")
            nc.sync.dma_start(xr[:], XQ[qt * 128:(qt + 1) * 128, :])
            qtsl = ds(qt * 128, 128)
            y = fw.tile([128, D], F32, name="y", tag="y")
            for nn in range(2):
                ps = fps.tile([128, 512], F32, name="fcps", tag="fcps")
                for ft in range(8):
                    nc.tensor.matmul(
                        ps[:], CTXT[ft][:, qtsl],
                        wfc[ft][:, nn * 512:(nn + 1) * 512],
                        start=(ft == 0), stop=(ft == 7))
                nc.vector.tensor_tensor(
                    out=y[:, nn * 512:(nn + 1) * 512], in0=ps[:],
                    in1=xr[:, nn * 512:(nn + 1) * 512], op=Alu.add)
            s1 = fw.tile([128, 1], F32, name="s1", tag="s1")
            nc.vector.reduce_sum(s1[:], y[:], axis=AxX)
            sqd = fw.tile([128, D], F32, name="sqd", tag="sqd")
            s2 = fw.tile([128, 1], F32, name="s2", tag="s2")
            nc.scalar.activation(sqd[:], y[:], Act.Square, accum_out=s2[:])
            mu = fw.tile([128, 1], F32, name="mu", tag="mu")
            nc.vector.tensor_scalar_mul(mu[:], s1[:], inv_d)
            ex2 = fw.tile([128, 1], F32, name="ex2", tag="ex2")
            nc.vector.tensor_scalar_mul(ex2[:], s2[:], inv_d)
            mu2 = fw.tile([128, 1], F32, name="mu2", tag="mu2")
            nc.vector.tensor_tensor(out=mu2[:], in0=mu[:], in1=mu[:],
                                    op=Alu.mult)
            var = fw.tile([128, 1], F32, name="var", tag="var")
            nc.vector.tensor_tensor(out=var[:], in0=ex2[:], in1=mu2[:],
                                    op=Alu.subtract)
            sd = fw.tile([128, 1], F32, name="sd", tag="sd")
            nc.scalar.activation(sd[:], var[:], Act.Sqrt, bias=epsb[:])
            rstd = fw.tile([128, 1], F32, name="rstd", tag="rstd")
            nc.vector.reciprocal(rstd[:], sd[:])
            o = fw.tile([128, D], F32, name="o", tag="o")
            nc.vector.tensor_scalar(
                out=o[:], in0=y[:], scalar1=mu[:], scalar2=rstd[:],
                op0=Alu.subtract, op1=Alu.mult)
            nc.sync.dma_start(OUT[qt * 128:(qt + 1) * 128, :], o[:])


_CACHED = {}


def _get_nc():
    if "nc" not in _CACHED:
        _CACHED["nc"] = build_nc(reps=1)
    return _CACHED["nc"]


def make_in_maps(input_Q, input_K, input_V, attn_mask, W_Q, W_K, W_V, W_fc):
    in_maps = []
    for c in range(NCORES):
        b = c // 4
        q0 = (c % 4) * QN
        in_maps.append({
            "XQ": np.ascontiguousarray(input_Q[b, q0:q0 + QN, :]),
            "XK": np.ascontiguousarray(input_K[b]),
            "XV": np.ascontiguousarray(input_V[b]),
            "MASKU8": np.ascontiguousarray(attn_mask[b, q0:q0 + QN, :]).view(np.uint8),
            "WQ": W_Q, "WK": W_K, "WV": W_V, "WFC": W_fc,
        })
    return in_maps


def assemble(results):
    out = np.empty((B, S, D), np.float32)
    attn = np.empty((B, H, S, S), np.float32)
    for c in range(NCORES):
        b = c // 4
        q0 = (c % 4) * QN
        out[b, q0:q0 + QN, :] = results[c]["OUT"]
        attn[b, :, q0:q0 + QN, :] = results[c]["ATTN"]
    return out, attn


def kernel(input_Q, input_K, input_V, attn_mask, W_Q, W_K, W_V, W_fc):
    input_Q = np.asarray(input_Q, dtype=np.float32)
    input_K = np.asarray(input_K, dtype=np.float32)
    input_V = np.asarray(input_V, dtype=np.float32)
    attn_mask = np.asarray(attn_mask)
    W_Q = np.ascontiguousarray(np.asarray(W_Q, dtype=np.float32))
    W_K = np.ascontiguousarray(np.asarray(W_K, dtype=np.float32))
    W_V = np.ascontiguousarray(np.asarray(W_V, dtype=np.float32))
    W_fc = np.ascontiguousarray(np.asarray(W_fc, dtype=np.float32))

    from concourse.bass_utils import run_bass_kernel_spmd
    nc = _get_nc()
    in_maps = make_in_maps(input_Q, input_K, input_V, attn_mask,
                           W_Q, W_K, W_V, W_fc)
    res = run_bass_kernel_spmd(nc, in_maps, core_ids=list(range(NCORES)))
    return assemble(res.results)
